# revision 11
# baseline (speedup 1.0000x reference)
"""Trainium2 Bass kernel for nn_EnsembleModel (histogram_binning).

Math:
  hist[p,q]  = sum_{b,i,j} [adds[b,i]==p] * a_arc[b,i,j] * [adds[b,j]==q]
  score      = sigmoid(hist)                                  # [50,50]
  out[b,i,j] = s_arc[b,i,j] + ALPHA * score[pos[b,i], pos[b,j]]

One-hot matmul formulation, all bf16 (one-hots exact; a/s rounded on host):

  phase 1 (per batch):  P[p,jblk] = sum_i U[i,p] A[i,j]  -- col-packed pairs
                        (jb0 -> PSUM rows 0-49, jb1 -> rows 64-113, concurrent)
                        PT chunks = PE-transpose of P (pipelined one batch
                        behind the P matmuls so ACT copies never stall PE)
                        hist     += PT.T @ U  -- col-packed jc pairs
  AllGather(hist shards) -> local DVE tree-sum -> sigmoid * ALPHA
  phase 2 (per batch):  sc2 [50,128] holds score twice (cols 0-49, 64-113)
                        gt2 [128,SL] = sc2.T @ vt (both replicas in one MM)
                        chunk pairs (c even, c+1): 4 MMs interleaved so the
                        row-group-0 and row-group-64 MMs run concurrently
                        s-add split 3 ways to balance engines:
                          direct DVE PSUM-add | ACT copy + DVE bf16 add |
                          ACT copy + GpSimd bf16 add

DMA schedule: u first, then a at full HBM rate (SP ring, 1MB half-batch
loads); vt2 and the s stream (ACT ring) are dependency-throttled behind the
a stream so the ~40us mesh-AllGather latency hides under the s-loads.
"""

import numpy as np
import ml_dtypes

ALPHA = 0.3
NP = 50          # n_pos
SL = 1024        # sequence length
BZ = 64          # global batch
NCORES = 8
B = BZ // NCORES  # local batch per core
NCH = SL // 128   # 128-row chunks per matrix
NBLK = SL // 512  # 512-col blocks per matrix

_CACHE = {}


def _build_nc():
    import concourse.bacc as bacc
    import concourse.mybir as mybir
    import concourse.tile as tile
    from concourse.tile import add_dep_helper

    f32 = mybir.dt.float32
    bf16 = mybir.dt.bfloat16
    fp8 = mybir.dt.float8e4
    nc = bacc.Bacc(
        "TRN2", target_bir_lowering=False, debug=False, num_devices=NCORES
    )

    a_d = nc.dram_tensor("a", [B, SL, SL], bf16, kind="ExternalInput")
    s_d = nc.dram_tensor("s", [B, SL, SL], bf16, kind="ExternalInput")
    u_d = nc.dram_tensor("u", [128, B, NCH, NP], fp8, kind="ExternalInput")
    vt_d = nc.dram_tensor("vt", [128, B, SL], bf16, kind="ExternalInput")
    eye_d = nc.dram_tensor("eye", [NP, NP], bf16, kind="ExternalInput")
    out_d = nc.dram_tensor("out", [B, SL, SL], bf16, kind="ExternalOutput")

    with tile.TileContext(nc) as tc:
        with (
            tc.tile_pool(name="const", bufs=1) as const_pool,
            tc.tile_pool(name="apool", bufs=3) as a_pool,
            tc.tile_pool(name="opool", bufs=6) as o_pool,
            tc.tile_pool(name="gpool", bufs=3) as g_pool,
            tc.tile_pool(name="ppool", bufs=2) as p_pool,
            tc.tile_pool(name="ptsb", bufs=8) as pt_pool,
            tc.tile_pool(name="gtsb", bufs=2) as gt_pool,
            tc.tile_pool(name="small", bufs=1) as small_pool,
            tc.tile_pool(name="dram", bufs=1, space="DRAM") as dram_pool,
        ):
            u_sb = const_pool.tile([128, B, NCH, NP], fp8)
            eye_sb = const_pool.tile([NP, NP], bf16)
            vt_sb = const_pool.tile([128, B, SL], bf16)
            s_sb = const_pool.tile([128, B, NCH, SL], bf16)
            nc.sync.dma_start(eye_sb[:], eye_d[:])
            nc.scalar.dma_start(u_sb[:], u_d[:])

            a_loads = []

            # ---- Phase 1: local histogram ----
            with (
                tc.tile_pool(name="histps", bufs=1, space="PSUM") as hist_pool,
                tc.tile_pool(name="pps", bufs=2, space="PSUM") as pps_pool,
                tc.tile_pool(name="tpps", bufs=2, space="PSUM") as tpps_pool,
            ):
                hist_ps = hist_pool.tile([128, NP], f32)
                p_hist = []  # (b, p_sb) pending transpose+hist work

                def emit_hist(b, p_sb):
                    # 8 transposes first, then col-packed hist MM pairs.
                    pts_l = []
                    for jc in range(NCH):
                        tp_ps = tpps_pool.tile([128, NP], bf16, tag="tp")
                        nc.tensor.transpose(
                            tp_ps[:], p_sb[:, jc * 128:(jc + 1) * 128],
                            eye_sb[:],
                        )
                        pts = pt_pool.tile([128, NP], bf16, tag="pts")
                        nc.vector.tensor_copy(pts[:], tp_ps[:])
                        pts_l.append(pts)
                    first = b == 0
                    last = b == B - 1
                    for jc in range(0, NCH, 2):
                        nc.tensor.matmul(
                            hist_ps[0:NP, :], pts_l[jc][:], u_sb[:, b, jc, :],
                            start=(first and jc == 0),
                            stop=(last and jc == NCH - 2),
                        )
                        nc.tensor.matmul(
                            hist_ps[64:64 + NP, :], pts_l[jc + 1][:],
                            u_sb[:, b, jc + 1, :],
                            start=(first and jc == 0),
                            stop=(last and jc == NCH - 2),
                            tile_position=(0, 64),
                        )

                for b in range(B):
                    # transpose+hist for the PREVIOUS batch first: these fill
                    # the PE while this batch's a-tiles are still loading.
                    if p_hist:
                        emit_hist(*p_hist.pop(0))
                    at_lo = a_pool.tile([128, 4, SL], bf16, tag="a")
                    at_hi = a_pool.tile([128, 4, SL], bf16, tag="a")
                    a_loads.append(nc.sync.dma_start(
                        at_lo[:],
                        a_d[b, 0:512, :].rearrange("(c p) j -> p c j", p=128),
                    ))
                    a_loads.append(nc.sync.dma_start(
                        at_hi[:],
                        a_d[b, 512:1024, :].rearrange("(c p) j -> p c j", p=128),
                    ))
                    p_ps = pps_pool.tile([128, 512], f32, tag="pp")
                    for ic in range(NCH):
                        at = at_lo if ic < 4 else at_hi
                        icc = ic % 4
                        st_, sp_ = (ic == 0), (ic == NCH - 1)
                        nc.tensor.matmul(
                            p_ps[0:NP, :],
                            u_sb[:, b, ic, :],
                            at[:, icc, 0:512],
                            start=st_, stop=sp_,
                        )
                        nc.tensor.matmul(
                            p_ps[64:64 + NP, :],
                            u_sb[:, b, ic, :],
                            at[:, icc, 512:1024],
                            start=st_, stop=sp_,
                            tile_position=(0, 64),
                        )
                    p_sb = p_pool.tile([NP, SL], bf16, tag="p")
                    nc.scalar.copy(p_sb[:, 0:512], p_ps[0:NP, :])
                    nc.scalar.copy(p_sb[:, 512:1024], p_ps[64:64 + NP, :])
                    p_hist.append((b, p_sb))
                emit_hist(*p_hist.pop(0))

                htmp = small_pool.tile([NP, NP], f32, tag="ht")
                nc.vector.tensor_copy(htmp[:], hist_ps[64:64 + NP, :])
                hist_sb = small_pool.tile([NP, NP], f32, tag="h0")
                nc.vector.tensor_tensor(
                    hist_sb[:], hist_ps[0:NP, :], htmp[:], mybir.AluOpType.add
                )

            # ---- vt2 + s loads: ACT ring, throttled so a keeps full BW
            # and s streams through the collective window.
            vld = nc.scalar.dma_start(vt_sb[:], vt_d[:])
            add_dep_helper(
                vld.ins, a_loads[15].ins, reason="vt2 after a stream"
            )
            for b in range(B):
                sld = nc.scalar.dma_start(
                    s_sb[:, b, :, :], s_d[b].rearrange("(c p) j -> p c j", p=128)
                )
                add_dep_helper(
                    sld.ins,
                    a_loads[14].ins,
                    reason="s-loads after a stream",
                )

            # ---- Tiny dummy AllGather issued first: absorbs any
            # first-collective setup cost while phase 1 still runs.
            ccw_in = dram_pool.tile([2, 2], f32, tag="ccwin")
            ccw_out = dram_pool.tile(
                [2 * NCORES, 2], f32, tag="ccwout", addr_space="Shared"
            )
            nc.gpsimd.collective_compute(
                "AllGather",
                mybir.AluOpType.bypass,
                replica_groups=[list(range(NCORES))],
                ins=[ccw_in.opt()],
                outs=[ccw_out.opt()],
            )

            # ---- AllGather partial hists + local sum + sigmoid ----
            cc_in = dram_pool.tile([NP, NP], f32, tag="ccin")
            cc_out = dram_pool.tile(
                [NCORES * NP, NP], f32, tag="ccout", addr_space="Shared"
            )
            nc.gpsimd.dma_start(cc_in[:], hist_sb[:])
            nc.gpsimd.collective_compute(
                "AllGather",
                mybir.AluOpType.bypass,
                replica_groups=[list(range(NCORES))],
                ins=[cc_in.opt()],
                outs=[cc_out.opt()],
            )
            hist8 = small_pool.tile([NP, NCORES, NP], f32, tag="h8")
            nc.gpsimd.dma_start(
                hist8[:], cc_out.opt().rearrange("(r p) q -> p r q", p=NP)
            )
            h4 = small_pool.tile([NP, 4, NP], f32, tag="h4")
            nc.vector.tensor_tensor(
                h4[:], hist8[:, 0:4, :], hist8[:, 4:8, :], mybir.AluOpType.add
            )
            h2 = small_pool.tile([NP, 2, NP], f32, tag="h2")
            nc.vector.tensor_tensor(
                h2[:], h4[:, 0:2, :], h4[:, 2:4, :], mybir.AluOpType.add
            )
            h1 = small_pool.tile([NP, NP], f32, tag="h1")
            nc.vector.tensor_tensor(
                h1[:], h2[:, 0, :], h2[:, 1, :], mybir.AluOpType.add
            )
            sc2 = small_pool.tile([NP, 128], bf16, tag="sc2")
            nc.vector.memset(sc2[:], 0.0)
            nc.scalar.activation(
                sc2[:, 0:NP], h1[:], mybir.ActivationFunctionType.Sigmoid
            )
            nc.scalar.activation(
                sc2[:, 64:64 + NP], h1[:], mybir.ActivationFunctionType.Sigmoid
            )
            nc.vector.tensor_scalar_mul(sc2[:], sc2[:], ALPHA)

            # ---- Phase 2 ----
            with (
                tc.tile_pool(name="gtps", bufs=1, space="PSUM") as gtps_pool,
                tc.tile_pool(name="ops", bufs=3, space="PSUM") as ops_pool,
            ):
                for b in range(B):
                    gt2 = gt_pool.tile([128, SL], bf16, tag="gt")
                    for ib in range(NBLK):
                        gt_ps = gtps_pool.tile([128, 512], f32, tag="gtp")
                        nc.tensor.matmul(
                            gt_ps[:],
                            sc2[:],
                            vt_sb[0:NP, b, ib * 512:(ib + 1) * 512],
                            start=True, stop=True,
                        )
                        if ib == 0:
                            nc.vector.tensor_copy(
                                gt2[:, ib * 512:(ib + 1) * 512], gt_ps[:]
                            )
                        else:
                            nc.scalar.copy(
                                gt2[:, ib * 512:(ib + 1) * 512], gt_ps[:]
                            )
                    # stage-sorted groups of 3 chunks: all MMs, then the
                    # PSUM-draining copies/adds, then stores.  Copies never
                    # queue behind adds of earlier chunks, so PSUM banks
                    # recycle promptly and the PE stays fed.
                    for g0 in (0, 2, 4, 6):
                        cs = [g0, g0 + 1]
                        pss = []
                        for ci in range(2):
                            o_ps = ops_pool.tile([128, SL], f32, tag="op")
                            pss.append(o_ps)
                        # jb-major emission: the row-group-0 and row-group-64
                        # MMs of the two chunks sit adjacent in the PE queue
                        # and execute concurrently.
                        for jb in range(NBLK):
                            sl_ = slice(jb * 512, (jb + 1) * 512)
                            for c, o_ps in zip(cs, pss):
                                lo = (c % 2) * 64
                                tp = {} if c % 2 == 0 else {
                                    "tile_position": (64, 0)
                                }
                                nc.tensor.matmul(
                                    o_ps[:, sl_],
                                    gt2[lo:lo + NP, c * 128:(c + 1) * 128],
                                    vt_sb[lo:lo + NP, b, sl_],
                                    start=True, stop=True,
                                    **tp,
                                )
                        ots = []
                        gsbs = []
                        for c, o_ps in zip(cs, pss):
                            m = (b * NCH + c) % 16
                            ot = o_pool.tile([128, SL], bf16, tag="o")
                            if m < 5:
                                gsbs.append(None)
                            else:
                                gsb = g_pool.tile([128, SL], bf16, tag="g")
                                nc.scalar.copy(gsb[:], o_ps[:])
                                gsbs.append(gsb)
                            ots.append(ot)
                        for c, o_ps, ot, gsb in zip(cs, pss, ots, gsbs):
                            m = (b * NCH + c) % 16
                            if gsb is None:
                                nc.vector.tensor_tensor(
                                    ot[:], s_sb[:, b, c, :], o_ps[:],
                                    mybir.AluOpType.add,
                                )
                            elif m < 12:
                                nc.vector.tensor_tensor(
                                    ot[:], s_sb[:, b, c, :], gsb[:],
                                    mybir.AluOpType.add,
                                )
                            else:
                                nc.gpsimd.tensor_tensor(
                                    ot[:], s_sb[:, b, c, :], gsb[:],
                                    mybir.AluOpType.add,
                                )
                        for c, ot in zip(cs, ots):
                            nc.sync.dma_start(
                                out_d[b, c * 128:(c + 1) * 128, :], ot[:]
                            )

    nc.compile()
    return nc


def _get_nc():
    if "nc" not in _CACHE:
        _CACHE["nc"] = _build_nc()
    return _CACHE["nc"]


def kernel(a_arc, s_arc, adds, pos, n_pos, _trace=False, _return_perf=False):
    from concourse.bass_utils import run_bass_kernel_spmd

    assert int(n_pos) == NP
    a = np.asarray(a_arc, dtype=np.float32)
    s = np.asarray(s_arc, dtype=np.float32)
    adds = np.asarray(adds)
    pos = np.asarray(pos)

    rng = np.arange(NP)
    eye = np.eye(NP, dtype=ml_dtypes.bfloat16)

    in_maps = []
    for k in range(NCORES):
        sl = slice(k * B, (k + 1) * B)
        adds_sh = adds[sl]
        pos_sh = pos[sl]
        # u[p, b, c, q] = [adds[b, c*128+p] == q]  (partition-major, fp8)
        u2 = (
            adds_sh.reshape(B, NCH, 128).transpose(2, 0, 1)[..., None] == rng
        ).astype(ml_dtypes.float8_e4m3)
        # vt2[p, b, i]: one-hot [pos==p] replicated at rows 0-49 and 64-113
        oh = (rng[:, None, None] == pos_sh[None, :, :]).astype(
            ml_dtypes.bfloat16
        )
        vt2 = np.zeros((128, B, SL), dtype=ml_dtypes.bfloat16)
        vt2[0:NP] = oh
        vt2[64:64 + NP] = oh
        in_maps.append(
            {
                "a": np.ascontiguousarray(a[sl]).astype(ml_dtypes.bfloat16),
                "s": np.ascontiguousarray(s[sl]).astype(ml_dtypes.bfloat16),
                "u": np.ascontiguousarray(u2),
                "vt": np.ascontiguousarray(vt2),
                "eye": eye,
            }
        )

    nc = _get_nc()
    res = run_bass_kernel_spmd(
        nc, in_maps, core_ids=list(range(NCORES)), trace=_trace
    )
    out = np.concatenate([r["out"] for r in res.results], axis=0).astype(np.float32)
    if _return_perf:
        return out, res
    return out


# revision 12
# speedup vs baseline: 1.0703x; 1.0703x over previous
"""Trainium2 Bass kernel for nn_EnsembleModel (histogram_binning).

Math:
  hist[p,q]  = sum_{b,i,j} [adds[b,i]==p] * a_arc[b,i,j] * [adds[b,j]==q]
  score      = sigmoid(hist)                                  # [50,50]
  out[b,i,j] = s_arc[b,i,j] + ALPHA * score[pos[b,i], pos[b,j]]

One-hot matmul formulation, all bf16 (one-hots exact; a/s rounded on host):

  phase 1 (per batch):  P[p,jblk] = sum_i U[i,p] A[i,j]  -- col-packed pairs
                        (jb0 -> PSUM rows 0-49, jb1 -> rows 64-113, concurrent)
                        PT chunks = PE-transpose of P (pipelined one batch
                        behind the P matmuls so ACT copies never stall PE)
                        hist     += PT.T @ U  -- col-packed jc pairs
  AllGather(hist shards) -> local DVE tree-sum -> sigmoid * ALPHA
  phase 2 (per batch):  sc2 [50,128] holds score twice (cols 0-49, 64-113)
                        gt2 [128,SL] = sc2.T @ vt (both replicas in one MM)
                        chunk pairs (c even, c+1): 4 MMs interleaved so the
                        row-group-0 and row-group-64 MMs run concurrently
                        s-add split 3 ways to balance engines:
                          direct DVE PSUM-add | ACT copy + DVE bf16 add |
                          ACT copy + GpSimd bf16 add

DMA schedule: u first, then a at full HBM rate (SP ring, 1MB half-batch
loads); vt2 and the s stream (ACT ring) are dependency-throttled behind the
a stream so the ~40us mesh-AllGather latency hides under the s-loads.
"""

import numpy as np
import ml_dtypes

ALPHA = 0.3
NP = 50          # n_pos
SL = 1024        # sequence length
BZ = 64          # global batch
NCORES = 8
B = BZ // NCORES  # local batch per core
NCH = SL // 128   # 128-row chunks per matrix
NBLK = SL // 512  # 512-col blocks per matrix

_CACHE = {}


def _build_nc():
    import concourse.bacc as bacc
    import concourse.mybir as mybir
    import concourse.tile as tile
    from concourse.tile import add_dep_helper

    f32 = mybir.dt.float32
    bf16 = mybir.dt.bfloat16
    fp8 = mybir.dt.float8e4
    nc = bacc.Bacc(
        "TRN2", target_bir_lowering=False, debug=False, num_devices=NCORES
    )

    a_d = nc.dram_tensor("a", [B, SL, SL], bf16, kind="ExternalInput")
    s_d = nc.dram_tensor("s", [B, SL, SL], bf16, kind="ExternalInput")
    u_d = nc.dram_tensor("u", [128, B, NCH, NP], fp8, kind="ExternalInput")
    vt_d = nc.dram_tensor("vt", [128, B, SL], bf16, kind="ExternalInput")
    eye_d = nc.dram_tensor("eye", [NP, NP], bf16, kind="ExternalInput")
    out_d = nc.dram_tensor("out", [B, SL, SL], bf16, kind="ExternalOutput")

    with tile.TileContext(nc) as tc:
        with (
            tc.tile_pool(name="const", bufs=1) as const_pool,
            tc.tile_pool(name="apool", bufs=3) as a_pool,
            tc.tile_pool(name="opool", bufs=6) as o_pool,
            tc.tile_pool(name="gpool", bufs=3) as g_pool,
            tc.tile_pool(name="ppool", bufs=2) as p_pool,
            tc.tile_pool(name="ptsb", bufs=8) as pt_pool,
            tc.tile_pool(name="gtsb", bufs=2) as gt_pool,
            tc.tile_pool(name="small", bufs=1) as small_pool,
            tc.tile_pool(name="dram", bufs=1, space="DRAM") as dram_pool,
        ):
            u_sb = const_pool.tile([128, B, NCH, NP], fp8)
            eye_sb = const_pool.tile([NP, NP], bf16)
            vt_sb = const_pool.tile([128, B, SL], bf16)
            s_sb = const_pool.tile([128, B, NCH, SL], bf16)
            nc.sync.dma_start(eye_sb[:], eye_d[:])
            nc.scalar.dma_start(u_sb[:], u_d[:])

            a_loads = []

            # ---- Phase 1: local histogram ----
            with (
                tc.tile_pool(name="histps", bufs=1, space="PSUM") as hist_pool,
                tc.tile_pool(name="pps", bufs=2, space="PSUM") as pps_pool,
                tc.tile_pool(name="tpps", bufs=2, space="PSUM") as tpps_pool,
            ):
                hist_ps = hist_pool.tile([128, NP], f32)
                p_hist = []  # (b, p_sb) pending transpose+hist work

                def emit_hist(b, p_sb):
                    # 8 transposes first, then col-packed hist MM pairs.
                    pts_l = []
                    for jc in range(NCH):
                        tp_ps = tpps_pool.tile([128, NP], bf16, tag="tp")
                        nc.tensor.transpose(
                            tp_ps[:], p_sb[:, jc * 128:(jc + 1) * 128],
                            eye_sb[:],
                        )
                        pts = pt_pool.tile([128, NP], bf16, tag="pts")
                        nc.vector.tensor_copy(pts[:], tp_ps[:])
                        pts_l.append(pts)
                    first = b == 0
                    last = b == B - 1
                    for jc in range(0, NCH, 2):
                        nc.tensor.matmul(
                            hist_ps[0:NP, :], pts_l[jc][:], u_sb[:, b, jc, :],
                            start=(first and jc == 0),
                            stop=(last and jc == NCH - 2),
                        )
                        nc.tensor.matmul(
                            hist_ps[64:64 + NP, :], pts_l[jc + 1][:],
                            u_sb[:, b, jc + 1, :],
                            start=(first and jc == 0),
                            stop=(last and jc == NCH - 2),
                            tile_position=(0, 64),
                        )

                for b in range(B):
                    # transpose+hist for the PREVIOUS batch first: these fill
                    # the PE while this batch's a-tiles are still loading.
                    if p_hist:
                        emit_hist(*p_hist.pop(0))
                    at_lo = a_pool.tile([128, 4, SL], bf16, tag="a")
                    at_hi = a_pool.tile([128, 4, SL], bf16, tag="a")
                    a_loads.append(nc.sync.dma_start(
                        at_lo[:],
                        a_d[b, 0:512, :].rearrange("(c p) j -> p c j", p=128),
                    ))
                    a_loads.append(nc.sync.dma_start(
                        at_hi[:],
                        a_d[b, 512:1024, :].rearrange("(c p) j -> p c j", p=128),
                    ))
                    p_ps = pps_pool.tile([128, 512], f32, tag="pp")
                    for ic in range(NCH):
                        at = at_lo if ic < 4 else at_hi
                        icc = ic % 4
                        st_, sp_ = (ic == 0), (ic == NCH - 1)
                        nc.tensor.matmul(
                            p_ps[0:NP, :],
                            u_sb[:, b, ic, :],
                            at[:, icc, 0:512],
                            start=st_, stop=sp_,
                        )
                        nc.tensor.matmul(
                            p_ps[64:64 + NP, :],
                            u_sb[:, b, ic, :],
                            at[:, icc, 512:1024],
                            start=st_, stop=sp_,
                            tile_position=(0, 64),
                        )
                    p_sb = p_pool.tile([NP, SL], bf16, tag="p")
                    nc.scalar.copy(p_sb[:, 0:512], p_ps[0:NP, :])
                    nc.scalar.copy(p_sb[:, 512:1024], p_ps[64:64 + NP, :])
                    p_hist.append((b, p_sb))
                emit_hist(*p_hist.pop(0))

                htmp = small_pool.tile([NP, NP], f32, tag="ht")
                nc.vector.tensor_copy(htmp[:], hist_ps[64:64 + NP, :])
                hist_sb = small_pool.tile([NP, NP], f32, tag="h0")
                nc.vector.tensor_tensor(
                    hist_sb[:], hist_ps[0:NP, :], htmp[:], mybir.AluOpType.add
                )

            # ---- vt2 + s loads: ACT ring, throttled so a keeps full BW
            # and s streams through the collective window.
            vld = nc.scalar.dma_start(vt_sb[:], vt_d[:])
            add_dep_helper(
                vld.ins, a_loads[15].ins, reason="vt2 after a stream"
            )
            for b in range(B):
                sld = nc.scalar.dma_start(
                    s_sb[:, b, :, :], s_d[b].rearrange("(c p) j -> p c j", p=128)
                )
                add_dep_helper(
                    sld.ins,
                    a_loads[14].ins,
                    reason="s-loads after a stream",
                )

            # ---- Tiny dummy AllGather issued first: absorbs any
            # first-collective setup cost while phase 1 still runs.
            ccw_in = dram_pool.tile([2, 2], f32, tag="ccwin")
            ccw_out = dram_pool.tile(
                [2 * NCORES, 2], f32, tag="ccwout", addr_space="Shared"
            )
            nc.gpsimd.collective_compute(
                "AllGather",
                mybir.AluOpType.bypass,
                replica_groups=[list(range(NCORES))],
                ins=[ccw_in.opt()],
                outs=[ccw_out.opt()],
            )

            # ---- AllGather partial hists + local sum + sigmoid ----
            cc_in = dram_pool.tile([NP, NP], f32, tag="ccin")
            cc_out = dram_pool.tile(
                [NCORES * NP, NP], f32, tag="ccout", addr_space="Shared"
            )
            nc.gpsimd.dma_start(cc_in[:], hist_sb[:])
            nc.gpsimd.collective_compute(
                "AllGather",
                mybir.AluOpType.bypass,
                replica_groups=[list(range(NCORES))],
                ins=[cc_in.opt()],
                outs=[cc_out.opt()],
            )
            hist8 = small_pool.tile([NP, NCORES, NP], f32, tag="h8")
            nc.gpsimd.dma_start(
                hist8[:], cc_out.opt().rearrange("(r p) q -> p r q", p=NP)
            )
            h4 = small_pool.tile([NP, 4, NP], f32, tag="h4")
            nc.vector.tensor_tensor(
                h4[:], hist8[:, 0:4, :], hist8[:, 4:8, :], mybir.AluOpType.add
            )
            h2 = small_pool.tile([NP, 2, NP], f32, tag="h2")
            nc.vector.tensor_tensor(
                h2[:], h4[:, 0:2, :], h4[:, 2:4, :], mybir.AluOpType.add
            )
            h1 = small_pool.tile([NP, NP], f32, tag="h1")
            nc.vector.tensor_tensor(
                h1[:], h2[:, 0, :], h2[:, 1, :], mybir.AluOpType.add
            )
            sc2 = small_pool.tile([NP, 128], bf16, tag="sc2")
            nc.vector.memset(sc2[:], 0.0)
            nc.scalar.activation(
                sc2[:, 0:NP], h1[:], mybir.ActivationFunctionType.Sigmoid
            )
            nc.scalar.activation(
                sc2[:, 64:64 + NP], h1[:], mybir.ActivationFunctionType.Sigmoid
            )
            nc.vector.tensor_scalar_mul(sc2[:], sc2[:], ALPHA)

            # ---- Phase 2 ----
            with (
                tc.tile_pool(name="ops", bufs=4, space="PSUM") as ops_pool,
            ):
                def emit_gt2(b):
                    gt2 = gt_pool.tile([128, SL], bf16, tag="gt")
                    for ib in range(NBLK):
                        gt_ps = ops_pool.tile([128, SL], f32, tag="op")
                        nc.tensor.matmul(
                            gt_ps[:, 0:512],
                            sc2[:],
                            vt_sb[0:NP, b, ib * 512:(ib + 1) * 512],
                            start=True, stop=True,
                        )
                        if ib == 0:
                            nc.vector.tensor_copy(
                                gt2[:, ib * 512:(ib + 1) * 512],
                                gt_ps[:, 0:512],
                            )
                        else:
                            nc.scalar.copy(
                                gt2[:, ib * 512:(ib + 1) * 512],
                                gt_ps[:, 0:512],
                            )
                    return gt2

                gt_q = [emit_gt2(0)]
                for b in range(B):
                    gt2 = gt_q.pop(0)
                    if b + 1 < B:
                        gt_q.append(emit_gt2(b + 1))
                    # stage-sorted groups of 3 chunks: all MMs, then the
                    # PSUM-draining copies/adds, then stores.  Copies never
                    # queue behind adds of earlier chunks, so PSUM banks
                    # recycle promptly and the PE stays fed.
                    for g0 in (0, 3, 6):
                        cs = list(range(g0, min(g0 + 3, NCH)))
                        pss = []
                        for ci in range(len(cs)):
                            o_ps = ops_pool.tile([128, SL], f32, tag="op")
                            pss.append(o_ps)
                        # jb-major emission: the row-group-0 and row-group-64
                        # MMs of the two chunks sit adjacent in the PE queue
                        # and execute concurrently.
                        for jb in range(NBLK):
                            sl_ = slice(jb * 512, (jb + 1) * 512)
                            for c, o_ps in zip(cs, pss):
                                lo = (c % 2) * 64
                                tp = {} if c % 2 == 0 else {
                                    "tile_position": (64, 0)
                                }
                                nc.tensor.matmul(
                                    o_ps[:, sl_],
                                    gt2[lo:lo + NP, c * 128:(c + 1) * 128],
                                    vt_sb[lo:lo + NP, b, sl_],
                                    start=True, stop=True,
                                    **tp,
                                )
                        ots = []
                        gsbs = []
                        for c, o_ps in zip(cs, pss):
                            m = (b * NCH + c) % 16
                            ot = o_pool.tile([128, SL], bf16, tag="o")
                            if m < 4:
                                gsbs.append(None)
                            else:
                                gsb = g_pool.tile([128, SL], bf16, tag="g")
                                nc.scalar.copy(gsb[:], o_ps[:])
                                gsbs.append(gsb)
                            ots.append(ot)
                        for c, o_ps, ot, gsb in zip(cs, pss, ots, gsbs):
                            m = (b * NCH + c) % 16
                            if gsb is None:
                                nc.vector.tensor_tensor(
                                    ot[:], s_sb[:, b, c, :], o_ps[:],
                                    mybir.AluOpType.add,
                                )
                            elif m < 14:
                                nc.vector.tensor_tensor(
                                    ot[:], s_sb[:, b, c, :], gsb[:],
                                    mybir.AluOpType.add,
                                )
                            else:
                                nc.gpsimd.tensor_tensor(
                                    ot[:], s_sb[:, b, c, :], gsb[:],
                                    mybir.AluOpType.add,
                                )
                        for c, ot in zip(cs, ots):
                            nc.sync.dma_start(
                                out_d[b, c * 128:(c + 1) * 128, :], ot[:]
                            )

    nc.compile()
    return nc


def _get_nc():
    if "nc" not in _CACHE:
        _CACHE["nc"] = _build_nc()
    return _CACHE["nc"]


def kernel(a_arc, s_arc, adds, pos, n_pos, _trace=False, _return_perf=False):
    from concourse.bass_utils import run_bass_kernel_spmd

    assert int(n_pos) == NP
    a = np.asarray(a_arc, dtype=np.float32)
    s = np.asarray(s_arc, dtype=np.float32)
    adds = np.asarray(adds)
    pos = np.asarray(pos)

    rng = np.arange(NP)
    eye = np.eye(NP, dtype=ml_dtypes.bfloat16)

    in_maps = []
    for k in range(NCORES):
        sl = slice(k * B, (k + 1) * B)
        adds_sh = adds[sl]
        pos_sh = pos[sl]
        # u[p, b, c, q] = [adds[b, c*128+p] == q]  (partition-major, fp8)
        u2 = (
            adds_sh.reshape(B, NCH, 128).transpose(2, 0, 1)[..., None] == rng
        ).astype(ml_dtypes.float8_e4m3)
        # vt2[p, b, i]: one-hot [pos==p] replicated at rows 0-49 and 64-113
        oh = (rng[:, None, None] == pos_sh[None, :, :]).astype(
            ml_dtypes.bfloat16
        )
        vt2 = np.zeros((128, B, SL), dtype=ml_dtypes.bfloat16)
        vt2[0:NP] = oh
        vt2[64:64 + NP] = oh
        in_maps.append(
            {
                "a": np.ascontiguousarray(a[sl]).astype(ml_dtypes.bfloat16),
                "s": np.ascontiguousarray(s[sl]).astype(ml_dtypes.bfloat16),
                "u": np.ascontiguousarray(u2),
                "vt": np.ascontiguousarray(vt2),
                "eye": eye,
            }
        )

    nc = _get_nc()
    res = run_bass_kernel_spmd(
        nc, in_maps, core_ids=list(range(NCORES)), trace=_trace
    )
    out = np.concatenate([r["out"] for r in res.results], axis=0).astype(np.float32)
    if _return_perf:
        return out, res
    return out


# revision 13
# speedup vs baseline: 1.0846x; 1.0133x over previous
"""Trainium2 Bass kernel for nn_EnsembleModel (histogram_binning).

Math:
  hist[p,q]  = sum_{b,i,j} [adds[b,i]==p] * a_arc[b,i,j] * [adds[b,j]==q]
  score      = sigmoid(hist)                                  # [50,50]
  out[b,i,j] = s_arc[b,i,j] + ALPHA * score[pos[b,i], pos[b,j]]

One-hot matmul formulation, all bf16 (one-hots exact; a/s rounded on host):

  phase 1 (per batch):  P[p,jblk] = sum_i U[i,p] A[i,j]  -- col-packed pairs
                        (jb0 -> PSUM rows 0-49, jb1 -> rows 64-113, concurrent)
                        PT chunks = PE-transpose of P (pipelined one batch
                        behind the P matmuls so ACT copies never stall PE)
                        hist     += PT.T @ U  -- col-packed jc pairs
  AllGather(hist shards) -> local DVE tree-sum -> sigmoid * ALPHA
  phase 2 (per batch):  sc2 [50,128] holds score twice (cols 0-49, 64-113)
                        gt2 [128,SL] = sc2.T @ vt (both replicas in one MM)
                        chunk pairs (c even, c+1): 4 MMs interleaved so the
                        row-group-0 and row-group-64 MMs run concurrently
                        s-add split 3 ways to balance engines:
                          direct DVE PSUM-add | ACT copy + DVE bf16 add |
                          ACT copy + GpSimd bf16 add

DMA schedule: u first, then a at full HBM rate (SP ring, 1MB half-batch
loads); vt2 and the s stream (ACT ring) are dependency-throttled behind the
a stream so the ~40us mesh-AllGather latency hides under the s-loads.
"""

import numpy as np
import ml_dtypes

ALPHA = 0.3
NP = 50          # n_pos
SL = 1024        # sequence length
BZ = 64          # global batch
NCORES = 8
B = BZ // NCORES  # local batch per core
NCH = SL // 128   # 128-row chunks per matrix
NBLK = SL // 512  # 512-col blocks per matrix

_CACHE = {}


def _build_nc():
    import concourse.bacc as bacc
    import concourse.mybir as mybir
    import concourse.tile as tile
    from concourse.tile import add_dep_helper

    f32 = mybir.dt.float32
    bf16 = mybir.dt.bfloat16
    fp8 = mybir.dt.float8e4
    nc = bacc.Bacc(
        "TRN2", target_bir_lowering=False, debug=False, num_devices=NCORES
    )

    a_d = nc.dram_tensor("a", [B, SL, SL], bf16, kind="ExternalInput")
    s_d = nc.dram_tensor("s", [B, SL, SL], bf16, kind="ExternalInput")
    u_d = nc.dram_tensor("u", [128, B, NCH, NP], fp8, kind="ExternalInput")
    vt_d = nc.dram_tensor("vt", [128, B, SL], bf16, kind="ExternalInput")
    eye_d = nc.dram_tensor("eye", [NP, NP], bf16, kind="ExternalInput")
    out_d = nc.dram_tensor("out", [B, SL, SL], bf16, kind="ExternalOutput")

    with tile.TileContext(nc) as tc:
        with (
            tc.tile_pool(name="const", bufs=1) as const_pool,
            tc.tile_pool(name="apool", bufs=3) as a_pool,
            tc.tile_pool(name="opool", bufs=6) as o_pool,
            tc.tile_pool(name="gpool", bufs=3) as g_pool,
            tc.tile_pool(name="ppool", bufs=2) as p_pool,
            tc.tile_pool(name="ptsb", bufs=8) as pt_pool,
            tc.tile_pool(name="gtsb", bufs=2) as gt_pool,
            tc.tile_pool(name="small", bufs=1) as small_pool,
            tc.tile_pool(name="dram", bufs=1, space="DRAM") as dram_pool,
        ):
            u_sb = const_pool.tile([128, B, NCH, NP], fp8)
            eye_sb = const_pool.tile([NP, NP], bf16)
            vt_sb = const_pool.tile([128, B, SL], bf16)
            s_sb = const_pool.tile([128, B, NCH, SL], bf16)
            nc.sync.dma_start(eye_sb[:], eye_d[:])
            nc.scalar.dma_start(u_sb[:], u_d[:])

            a_loads = []

            # ---- Phase 1: local histogram ----
            with (
                tc.tile_pool(name="histps", bufs=1, space="PSUM") as hist_pool,
                tc.tile_pool(name="pps", bufs=2, space="PSUM") as pps_pool,
                tc.tile_pool(name="tpps", bufs=2, space="PSUM") as tpps_pool,
            ):
                hist_ps = hist_pool.tile([128, NP], f32)
                p_hist = []  # (b, p_sb) pending transpose+hist work

                def emit_hist(b, p_sb):
                    # 8 transposes first, then col-packed hist MM pairs.
                    pts_l = []
                    for jc in range(NCH):
                        tp_ps = tpps_pool.tile([128, NP], bf16, tag="tp")
                        nc.tensor.transpose(
                            tp_ps[:], p_sb[:, jc * 128:(jc + 1) * 128],
                            eye_sb[:],
                        )
                        pts = pt_pool.tile([128, NP], bf16, tag="pts")
                        nc.vector.tensor_copy(pts[:], tp_ps[:])
                        pts_l.append(pts)
                    first = b == 0
                    last = b == B - 1
                    for jc in range(0, NCH, 2):
                        nc.tensor.matmul(
                            hist_ps[0:NP, :], pts_l[jc][:], u_sb[:, b, jc, :],
                            start=(first and jc == 0),
                            stop=(last and jc == NCH - 2),
                        )
                        nc.tensor.matmul(
                            hist_ps[64:64 + NP, :], pts_l[jc + 1][:],
                            u_sb[:, b, jc + 1, :],
                            start=(first and jc == 0),
                            stop=(last and jc == NCH - 2),
                            tile_position=(0, 64),
                        )

                for b in range(B):
                    # transpose+hist for the PREVIOUS batch first: these fill
                    # the PE while this batch's a-tiles are still loading.
                    if p_hist:
                        emit_hist(*p_hist.pop(0))
                    at_lo = a_pool.tile([128, 4, SL], bf16, tag="a")
                    at_hi = a_pool.tile([128, 4, SL], bf16, tag="a")
                    a_loads.append(nc.sync.dma_start(
                        at_lo[:],
                        a_d[b, 0:512, :].rearrange("(c p) j -> p c j", p=128),
                    ))
                    a_loads.append(nc.sync.dma_start(
                        at_hi[:],
                        a_d[b, 512:1024, :].rearrange("(c p) j -> p c j", p=128),
                    ))
                    p_ps = pps_pool.tile([128, 512], f32, tag="pp")
                    for ic in range(NCH):
                        at = at_lo if ic < 4 else at_hi
                        icc = ic % 4
                        st_, sp_ = (ic == 0), (ic == NCH - 1)
                        nc.tensor.matmul(
                            p_ps[0:NP, :],
                            u_sb[:, b, ic, :],
                            at[:, icc, 0:512],
                            start=st_, stop=sp_,
                        )
                        nc.tensor.matmul(
                            p_ps[64:64 + NP, :],
                            u_sb[:, b, ic, :],
                            at[:, icc, 512:1024],
                            start=st_, stop=sp_,
                            tile_position=(0, 64),
                        )
                    p_sb = p_pool.tile([NP, SL], bf16, tag="p")
                    nc.scalar.copy(p_sb[:, 0:512], p_ps[0:NP, :])
                    nc.scalar.copy(p_sb[:, 512:1024], p_ps[64:64 + NP, :])
                    p_hist.append((b, p_sb))
                emit_hist(*p_hist.pop(0))

                htmp = small_pool.tile([NP, NP], f32, tag="ht")
                nc.vector.tensor_copy(htmp[:], hist_ps[64:64 + NP, :])
                hist_sb = small_pool.tile([NP, NP], f32, tag="h0")
                nc.vector.tensor_tensor(
                    hist_sb[:], hist_ps[0:NP, :], htmp[:], mybir.AluOpType.add
                )

            # ---- vt2 + s loads: ACT ring, throttled so a keeps full BW
            # and s streams through the collective window.
            vld = nc.scalar.dma_start(vt_sb[:], vt_d[:])
            add_dep_helper(
                vld.ins, a_loads[15].ins, reason="vt2 after a stream"
            )
            for b in range(B):
                sld = nc.scalar.dma_start(
                    s_sb[:, b, :, :], s_d[b].rearrange("(c p) j -> p c j", p=128)
                )
                add_dep_helper(
                    sld.ins,
                    a_loads[14].ins,
                    reason="s-loads after a stream",
                )

            # ---- Tiny dummy AllGather issued first: absorbs any
            # first-collective setup cost while phase 1 still runs.
            ccw_in = dram_pool.tile([2, 2], f32, tag="ccwin")
            ccw_out = dram_pool.tile(
                [2 * NCORES, 2], f32, tag="ccwout", addr_space="Shared"
            )
            nc.gpsimd.collective_compute(
                "AllGather",
                mybir.AluOpType.bypass,
                replica_groups=[list(range(NCORES))],
                ins=[ccw_in.opt()],
                outs=[ccw_out.opt()],
            )

            # ---- AllGather partial hists + local sum + sigmoid ----
            cc_in = dram_pool.tile([NP, NP], f32, tag="ccin")
            cc_out = dram_pool.tile(
                [NCORES * NP, NP], f32, tag="ccout", addr_space="Shared"
            )
            nc.gpsimd.dma_start(cc_in[:], hist_sb[:])
            nc.gpsimd.collective_compute(
                "AllGather",
                mybir.AluOpType.bypass,
                replica_groups=[list(range(NCORES))],
                ins=[cc_in.opt()],
                outs=[cc_out.opt()],
            )
            hist8 = small_pool.tile([NP, NCORES, NP], f32, tag="h8")
            nc.gpsimd.dma_start(
                hist8[:], cc_out.opt().rearrange("(r p) q -> p r q", p=NP)
            )
            h4 = small_pool.tile([NP, 4, NP], f32, tag="h4")
            nc.vector.tensor_tensor(
                h4[:], hist8[:, 0:4, :], hist8[:, 4:8, :], mybir.AluOpType.add
            )
            h2 = small_pool.tile([NP, 2, NP], f32, tag="h2")
            nc.vector.tensor_tensor(
                h2[:], h4[:, 0:2, :], h4[:, 2:4, :], mybir.AluOpType.add
            )
            h1 = small_pool.tile([NP, NP], f32, tag="h1")
            nc.vector.tensor_tensor(
                h1[:], h2[:, 0, :], h2[:, 1, :], mybir.AluOpType.add
            )
            sc2 = small_pool.tile([NP, 128], bf16, tag="sc2")
            nc.vector.memset(sc2[:], 0.0)
            nc.scalar.activation(
                sc2[:, 0:NP], h1[:], mybir.ActivationFunctionType.Sigmoid
            )
            nc.scalar.activation(
                sc2[:, 64:64 + NP], h1[:], mybir.ActivationFunctionType.Sigmoid
            )
            nc.vector.tensor_scalar_mul(sc2[:], sc2[:], ALPHA)

            # ---- Phase 2 ----
            with (
                tc.tile_pool(name="ops", bufs=4, space="PSUM") as ops_pool,
            ):
                def emit_gt2(b):
                    gt2 = gt_pool.tile([128, SL], bf16, tag="gt")
                    for ib in range(NBLK):
                        gt_ps = ops_pool.tile([128, SL], f32, tag="op")
                        nc.tensor.matmul(
                            gt_ps[:, 0:512],
                            sc2[:],
                            vt_sb[0:NP, b, ib * 512:(ib + 1) * 512],
                            start=True, stop=True,
                        )
                        if ib == 0:
                            nc.vector.tensor_copy(
                                gt2[:, ib * 512:(ib + 1) * 512],
                                gt_ps[:, 0:512],
                            )
                        else:
                            nc.scalar.copy(
                                gt2[:, ib * 512:(ib + 1) * 512],
                                gt_ps[:, 0:512],
                            )
                    return gt2

                gt_q = [emit_gt2(0)]
                for b in range(B):
                    gt2 = gt_q.pop(0)
                    if b + 1 < B:
                        gt_q.append(emit_gt2(b + 1))
                    # stage-sorted groups of 3 chunks: all MMs, then the
                    # PSUM-draining copies/adds, then stores.  Copies never
                    # queue behind adds of earlier chunks, so PSUM banks
                    # recycle promptly and the PE stays fed.
                    for g0 in (0, 3, 6):
                        cs = list(range(g0, min(g0 + 3, NCH)))
                        pss = []
                        for ci in range(len(cs)):
                            o_ps = ops_pool.tile([128, SL], f32, tag="op")
                            pss.append(o_ps)
                        # jb-major emission: the row-group-0 and row-group-64
                        # MMs of the two chunks sit adjacent in the PE queue
                        # and execute concurrently.
                        for jb in range(NBLK):
                            sl_ = slice(jb * 512, (jb + 1) * 512)
                            for c, o_ps in zip(cs, pss):
                                lo = (c % 2) * 64
                                tp = {} if c % 2 == 0 else {
                                    "tile_position": (64, 0)
                                }
                                nc.tensor.matmul(
                                    o_ps[:, sl_],
                                    gt2[lo:lo + NP, c * 128:(c + 1) * 128],
                                    vt_sb[lo:lo + NP, b, sl_],
                                    start=True, stop=True,
                                    **tp,
                                )
                        ots = []
                        gsbs = []
                        for c, o_ps in zip(cs, pss):
                            ot = o_pool.tile([128, SL], bf16, tag="o")
                            if c in (1, 5):
                                gsbs.append(None)
                            else:
                                gsb = g_pool.tile([128, SL], bf16, tag="g")
                                nc.scalar.copy(gsb[:], o_ps[:])
                                gsbs.append(gsb)
                            ots.append(ot)
                        for c, o_ps, ot, gsb in zip(cs, pss, ots, gsbs):
                            if gsb is None:
                                # direct PSUM add on DVE (chunks 1, 5)
                                nc.vector.tensor_tensor(
                                    ot[:], s_sb[:, b, c, :], o_ps[:],
                                    mybir.AluOpType.add,
                                )
                            elif c in (0, 4):
                                # GpSimd add early in each batch so its
                                # latency overlaps the rest of the batch
                                nc.gpsimd.tensor_tensor(
                                    ot[:], s_sb[:, b, c, :], gsb[:],
                                    mybir.AluOpType.add,
                                )
                            else:
                                nc.vector.tensor_tensor(
                                    ot[:], s_sb[:, b, c, :], gsb[:],
                                    mybir.AluOpType.add,
                                )
                        for c, ot in zip(cs, ots):
                            nc.sync.dma_start(
                                out_d[b, c * 128:(c + 1) * 128, :], ot[:]
                            )

    nc.compile()
    return nc


def _get_nc():
    if "nc" not in _CACHE:
        _CACHE["nc"] = _build_nc()
    return _CACHE["nc"]


def kernel(a_arc, s_arc, adds, pos, n_pos, _trace=False, _return_perf=False):
    from concourse.bass_utils import run_bass_kernel_spmd

    assert int(n_pos) == NP
    a = np.asarray(a_arc, dtype=np.float32)
    s = np.asarray(s_arc, dtype=np.float32)
    adds = np.asarray(adds)
    pos = np.asarray(pos)

    rng = np.arange(NP)
    eye = np.eye(NP, dtype=ml_dtypes.bfloat16)

    in_maps = []
    for k in range(NCORES):
        sl = slice(k * B, (k + 1) * B)
        adds_sh = adds[sl]
        pos_sh = pos[sl]
        # u[p, b, c, q] = [adds[b, c*128+p] == q]  (partition-major, fp8)
        u2 = (
            adds_sh.reshape(B, NCH, 128).transpose(2, 0, 1)[..., None] == rng
        ).astype(ml_dtypes.float8_e4m3)
        # vt2[p, b, i]: one-hot [pos==p] replicated at rows 0-49 and 64-113
        oh = (rng[:, None, None] == pos_sh[None, :, :]).astype(
            ml_dtypes.bfloat16
        )
        vt2 = np.zeros((128, B, SL), dtype=ml_dtypes.bfloat16)
        vt2[0:NP] = oh
        vt2[64:64 + NP] = oh
        in_maps.append(
            {
                "a": np.ascontiguousarray(a[sl]).astype(ml_dtypes.bfloat16),
                "s": np.ascontiguousarray(s[sl]).astype(ml_dtypes.bfloat16),
                "u": np.ascontiguousarray(u2),
                "vt": np.ascontiguousarray(vt2),
                "eye": eye,
            }
        )

    nc = _get_nc()
    res = run_bass_kernel_spmd(
        nc, in_maps, core_ids=list(range(NCORES)), trace=_trace
    )
    out = np.concatenate([r["out"] for r in res.results], axis=0).astype(np.float32)
    if _return_perf:
        return out, res
    return out
